# revision 10
# baseline (speedup 1.0000x reference)
"""A-Connect conv kernel for TRN2, data-parallel over batch on 8 NeuronCores.

Computation (per sample b):
    Z[b] = conv2d(X[b], W * Werr[b], SAME) + bias * Berr[b]; out = relu(Z)

Mapping: batch 32 -> 4 samples per core. Per sample the conv uses 1D
Winograd F(2,3) along image columns: for each output column pair the 3
column taps become 4 transformed taps shared by both outputs, cutting
tensor-engine work to 2/3 of the direct method (96 vs 144 N=512
matmuls per sample). The input transform V = B^T d (over column
windows) and weight transform U = G (W*Werr[b]) (over the column tap
axis) are precomputed on the host in fp32 and shipped as bf16. On
device, for each (F-half, 16-row chunk) the 4 Winograd components
M_j[F,512] = sum_dy U[dy,j]^T V_j[rows+dy] accumulate in 4 PSUM banks
(3 matmuls each, weights stationary so LDWEIGHTS hides under the
previous matmul's N=512 stream). The inverse transform
Y0 = M0+M1+M2, Y1 = M1-M2-M3 runs on the vector engine with the
per-sample bias folded into the first add (scalar_tensor_tensor);
relu + bf16 downcast runs on the scalar engine. Outputs are stored
F-major as bf16 and reassembled/upcast to NHWC fp32 on the host.
Measured rel err vs the fp32 reference: ~4e-3 (gate 2e-2).
"""

import numpy as np
import ml_dtypes

B, H, Wd, Cin, F, KH, KW = 32, 64, 64, 128, 256, 3, 3
NCORES = 8
BPC = B // NCORES  # samples per core
NP = Wd // 2  # 32 column pairs
RP = H + 2  # 66 padded rows
VROW = RP * NP  # 2112 positions per Winograd j-plane
NCH = 4  # chunks of 16 output rows (16*32 = 512 positions)
CHP = 512  # positions per chunk = one full PSUM bank of fp32

_compiled = None  # cached Bass program so repeated kernel() calls reuse it


def _build_bass():
    from concourse import bacc, tile, mybir

    nc = bacc.Bacc("TRN2", target_bir_lowering=False, debug=False)
    bf16 = mybir.dt.bfloat16
    f32 = mybir.dt.float32

    # V: input transform planes [Cin, j, padded-row, pair] flattened
    vp = nc.dram_tensor("vp", [BPC, Cin, 4 * VROW], bf16, kind="ExternalInput")
    # U: weight transform [Cin, dy, j, F]
    wm = nc.dram_tensor("wm", [BPC, Cin, KH * 4 * F], bf16, kind="ExternalInput")
    # per-sample bias, F on partitions: [128, half]
    mb = nc.dram_tensor("mb", [BPC, 128, 2], f32, kind="ExternalInput")
    # out: [half, chunk, k(col parity), F-half, pos]
    y = nc.dram_tensor("y", [BPC, 2, NCH, 2, 128, CHP], bf16, kind="ExternalOutput")

    with tile.TileContext(nc) as tc:
        with (
            tc.tile_pool(name="vpool", bufs=2) as vpool,
            tc.tile_pool(name="upool", bufs=2) as upool,
            tc.tile_pool(name="bpool", bufs=2) as bpool,
            tc.tile_pool(name="tpool", bufs=2) as tpool,
            tc.tile_pool(name="opool", bufs=8) as opool,
            tc.tile_pool(name="cpool", bufs=1) as cpool,
            tc.tile_pool(name="pspool", bufs=8, space="PSUM") as pspool,
        ):
            # PE warmup: dependency-free matmuls release the HAM clock
            # gate (~6.8us worst case) while the first input DMAs fly
            wu_in = cpool.tile([128, 512], bf16)
            nc.vector.memset(wu_in[:], 0.0)
            wu_ps = pspool.tile([128, 512], f32, tag="m")
            for i in range(20):
                nc.tensor.matmul(
                    wu_ps[:],
                    wu_in[:, :128],
                    wu_in[:],
                    start=(i == 0),
                    stop=(i == 19),
                )
            for b in range(BPC):
                ut = upool.tile([Cin, KH * 4 * F], bf16)
                nc.sync.dma_start(ut[:], wm[b])
                vt = vpool.tile([Cin, 4 * VROW], bf16)
                nc.sync.dma_start(vt[:], vp[b])
                bt = bpool.tile([128, 2], f32)
                nc.sync.dma_start(bt[:], mb[b])
                for h in range(2):
                    for c in range(NCH):
                        ms = []
                        for j in range(4):
                            ps = pspool.tile([128, CHP], f32, tag="m")
                            ms.append(ps)
                            for dy in range(KH):
                                uoff = (dy * 4 + j) * F + h * 128
                                voff = j * VROW + (16 * c + dy) * NP
                                nc.tensor.matmul(
                                    ps[:],
                                    ut[:, uoff : uoff + 128],
                                    vt[:, voff : voff + CHP],
                                    start=(dy == 0),
                                    stop=(dy == KH - 1),
                                )
                        # inverse transform: DVE is the bottleneck engine
                        # (cayman errata: ~620ns per [128,512] fp32 op), so
                        # it only runs the 4 unavoidable tensor-tensor adds;
                        # PSUM->SBUF bias copies and relus go to ScalarE and
                        # GpSimd. Each op reads at most one PSUM operand.
                        bias = bt[:, h : h + 1]
                        c0 = tpool.tile([128, CHP], f32)
                        nc.scalar.add(c0[:], ms[0][:], bias)
                        c1 = tpool.tile([128, CHP], f32)
                        nc.scalar.add(c1[:], ms[1][:], bias)
                        t1 = tpool.tile([128, CHP], f32)
                        nc.vector.tensor_add(t1[:], c0[:], ms[1][:])
                        t2 = tpool.tile([128, CHP], f32)
                        nc.vector.tensor_add(t2[:], t1[:], ms[2][:])
                        o0 = opool.tile([128, CHP], bf16)
                        nc.scalar.activation(
                            o0[:], t2[:], mybir.ActivationFunctionType.Relu
                        )
                        nc.sync.dma_start(y[b, h, c, 0], o0[:])
                        u1 = tpool.tile([128, CHP], f32)
                        nc.vector.tensor_sub(u1[:], c1[:], ms[2][:])
                        u2 = tpool.tile([128, CHP], f32)
                        nc.vector.tensor_sub(u2[:], u1[:], ms[3][:])
                        o1 = opool.tile([128, CHP], bf16)
                        nc.gpsimd.tensor_relu(o1[:], u2[:])
                        nc.sync.dma_start(y[b, h, c, 1], o1[:])
    nc.compile()
    return nc


def _prep_inputs(X, W, bias, Werr, Berr):
    bf = ml_dtypes.bfloat16
    X, W, bias, Werr, Berr = (np.asarray(a) for a in (X, W, bias, Werr, Berr))
    # weight transform U = G @ memW over the column-tap axis
    memW = W[None] * Werr  # [B, KH, KW, Cin, F]
    G = np.array(
        [[1, 0, 0], [0.5, 0.5, 0.5], [0.5, -0.5, 0.5], [0, 0, 1]], np.float32
    )
    U = np.einsum("jm,bdmcf->bcdjf", G, memW)  # [B, Cin, dy, j, F]
    U = np.ascontiguousarray(U, dtype=bf).reshape(B, Cin, KH * 4 * F)
    # input transform V_j over column windows of the zero-padded image
    Xp = np.zeros((B, RP, Wd + 2, Cin), np.float32)
    Xp[:, 1 : H + 1, 1 : Wd + 1] = X
    d0 = Xp[:, :, 0 : 2 * NP : 2]
    d1 = Xp[:, :, 1 : 2 * NP : 2]
    d2 = Xp[:, :, 2 : 2 * NP + 2 : 2]
    d3 = Xp[:, :, 3 : 2 * NP + 3 : 2]
    V = np.stack([d0 - d2, d1 + d2, d2 - d1, d1 - d3], axis=1)  # [B,4,RP,NP,Cin]
    V = np.ascontiguousarray(V.transpose(0, 4, 1, 2, 3), dtype=bf)
    V = V.reshape(B, Cin, 4 * VROW)
    # bias with F on partitions: mb[b, f', half]
    mbias = (bias[None] * Berr).astype(np.float32)  # [B, F]
    mbias = np.ascontiguousarray(mbias.reshape(B, 2, 128).transpose(0, 2, 1))
    return V, U, mbias


def _postprocess(y_cores):
    # y per core: [BPC, half, chunk, k, F-half, 16 rows, 32 pairs]
    out = np.concatenate(y_cores, axis=0)  # [B, 2, NCH, 2, 128, CHP]
    out = out.reshape(B, 2, NCH, 2, 128, 16, NP).astype(np.float32)
    # out[b, 16c+r, 2p+k, 128h+f] = y[b, h, c, k, f, r, p]
    out = out.transpose(0, 2, 5, 6, 3, 1, 4).reshape(B, H, Wd, F)
    return np.ascontiguousarray(out)


def kernel(X, W, bias, Werr, Berr):
    global _compiled
    from concourse.bass_utils import run_bass_kernel_spmd

    if _compiled is None:
        _compiled = _build_bass()
    nc = _compiled

    V, U, mbias = _prep_inputs(X, W, bias, Werr, Berr)
    in_maps = [
        {
            "vp": V[c * BPC : (c + 1) * BPC],
            "wm": U[c * BPC : (c + 1) * BPC],
            "mb": mbias[c * BPC : (c + 1) * BPC],
        }
        for c in range(NCORES)
    ]
    res = run_bass_kernel_spmd(nc, in_maps, core_ids=list(range(NCORES)))
    return _postprocess([r["y"] for r in res.results])


# revision 11
# speedup vs baseline: 2.8982x; 2.8982x over previous
"""A-Connect conv kernel for TRN2, data-parallel over batch on 8 NeuronCores.

Computation (per sample b):
    Z[b] = conv2d(X[b], W * Werr[b], SAME) + bias * Berr[b]; out = relu(Z)

Mapping: batch 32 -> 4 samples per core. Per sample the conv uses 1D
Winograd F(2,3) along image columns: for each output column pair the 3
column taps become 4 transformed taps shared by both outputs, cutting
tensor-engine work to 2/3 of the direct method (96 vs 144 N=512
matmuls per sample). The input transform V = B^T d (over column
windows) and weight transform U = G (W*Werr[b]) (over the column tap
axis) are precomputed on the host in fp32 and shipped as bf16. On
device, for each (F-half, 16-row chunk) the 4 Winograd components
M_j[F,512] = sum_dy U[dy,j]^T V_j[rows+dy] accumulate in 4 PSUM banks
(3 matmuls each, weights stationary so LDWEIGHTS hides under the
previous matmul's N=512 stream). The inverse transform
Y0 = M0+M1+M2, Y1 = M1-M2-M3 runs on the vector engine with the
per-sample bias folded into the first add (scalar_tensor_tensor);
relu + bf16 downcast runs on the scalar engine. Outputs are stored
F-major as bf16 and reassembled/upcast to NHWC fp32 on the host.
Measured rel err vs the fp32 reference: ~4e-3 (gate 2e-2).
"""

import numpy as np
import ml_dtypes

B, H, Wd, Cin, F, KH, KW = 32, 64, 64, 128, 256, 3, 3
NCORES = 8
BPC = B // NCORES  # samples per core
NP = Wd // 2  # 32 column pairs
RP = H + 2  # 66 padded rows
VROW = RP * NP  # 2112 positions per Winograd j-plane
NCH = 4  # chunks of 16 output rows (16*32 = 512 positions)
CHP = 512  # positions per chunk = one full PSUM bank of fp32

_compiled = None  # cached Bass program so repeated kernel() calls reuse it


def _build_bass():
    from concourse import bacc, tile, mybir

    nc = bacc.Bacc("TRN2", target_bir_lowering=False, debug=False)
    bf16 = mybir.dt.bfloat16
    f32 = mybir.dt.float32

    # V: input transform planes [Cin, j, padded-row, pair] flattened
    vp = nc.dram_tensor("vp", [BPC, Cin, 4 * VROW], bf16, kind="ExternalInput")
    # U: weight transform [Cin, dy, j, F]
    wm = nc.dram_tensor("wm", [BPC, Cin, KH * 4 * F], bf16, kind="ExternalInput")
    # per-sample bias, F on partitions: [128, half]
    mb = nc.dram_tensor("mb", [BPC, 128, 2], f32, kind="ExternalInput")
    # out: [half, chunk, k(col parity), F-half, pos]
    y = nc.dram_tensor("y", [BPC, 2, NCH, 2, 128, CHP], bf16, kind="ExternalOutput")

    with tile.TileContext(nc) as tc:
        with (
            tc.tile_pool(name="vpool", bufs=2) as vpool,
            tc.tile_pool(name="upool", bufs=2) as upool,
            tc.tile_pool(name="bpool", bufs=2) as bpool,
            tc.tile_pool(name="tpool", bufs=2) as tpool,
            tc.tile_pool(name="opool", bufs=8) as opool,
            tc.tile_pool(name="cpool", bufs=1) as cpool,
            tc.tile_pool(name="pspool", bufs=8, space="PSUM") as pspool,
        ):
            # PE warmup: dependency-free matmuls release the HAM clock
            # gate (~6.8us worst case) while the first input DMAs fly
            wu_in = cpool.tile([128, 512], bf16)
            nc.vector.memset(wu_in[:], 0.0)
            wu_ps = pspool.tile([128, 512], f32, tag="m")
            for i in range(20):
                nc.tensor.matmul(
                    wu_ps[:],
                    wu_in[:, :128],
                    wu_in[:],
                    start=(i == 0),
                    stop=(i == 19),
                )
            for b in range(BPC):
                ut = upool.tile([Cin, KH * 4 * F], bf16)
                nc.sync.dma_start(ut[:], wm[b])
                vt = vpool.tile([Cin, 4 * VROW], bf16)
                nc.sync.dma_start(vt[:], vp[b])
                bt = bpool.tile([128, 2], f32)
                nc.sync.dma_start(bt[:], mb[b])
                for h in range(2):
                    for c in range(NCH):
                        ms = []
                        for j in range(4):
                            ps = pspool.tile([128, CHP], f32, tag="m")
                            ms.append(ps)
                            for dy in range(KH):
                                uoff = (dy * 4 + j) * F + h * 128
                                voff = j * VROW + (16 * c + dy) * NP
                                nc.tensor.matmul(
                                    ps[:],
                                    ut[:, uoff : uoff + 128],
                                    vt[:, voff : voff + CHP],
                                    start=(dy == 0),
                                    stop=(dy == KH - 1),
                                )
                        # inverse transform: DVE is the bottleneck engine
                        # (cayman errata: ~620ns per [128,512] fp32 op), so
                        # it only runs the 4 unavoidable tensor-tensor adds;
                        # PSUM->SBUF bias copies and relus go to ScalarE and
                        # GpSimd. Each op reads at most one PSUM operand.
                        bias = bt[:, h : h + 1]
                        c0 = tpool.tile([128, CHP], f32)
                        nc.scalar.add(c0[:], ms[0][:], bias)
                        c1 = tpool.tile([128, CHP], f32)
                        nc.scalar.add(c1[:], ms[1][:], bias)
                        t1 = tpool.tile([128, CHP], f32)
                        nc.vector.tensor_add(t1[:], c0[:], ms[1][:])
                        t2 = tpool.tile([128, CHP], f32)
                        nc.vector.tensor_add(t2[:], t1[:], ms[2][:])
                        o0 = opool.tile([128, CHP], bf16)
                        nc.scalar.activation(
                            o0[:], t2[:], mybir.ActivationFunctionType.Relu
                        )
                        nc.sync.dma_start(y[b, h, c, 0], o0[:])
                        u1 = tpool.tile([128, CHP], f32)
                        nc.vector.tensor_sub(u1[:], c1[:], ms[2][:])
                        u2 = tpool.tile([128, CHP], f32)
                        nc.vector.tensor_sub(u2[:], u1[:], ms[3][:])
                        o1 = opool.tile([128, CHP], bf16)
                        nc.scalar.activation(
                            o1[:], u2[:], mybir.ActivationFunctionType.Relu
                        )
                        nc.sync.dma_start(y[b, h, c, 1], o1[:])
    nc.compile()
    return nc


def _prep_inputs(X, W, bias, Werr, Berr):
    bf = ml_dtypes.bfloat16
    X, W, bias, Werr, Berr = (np.asarray(a) for a in (X, W, bias, Werr, Berr))
    # weight transform U = G @ memW over the column-tap axis
    memW = W[None] * Werr  # [B, KH, KW, Cin, F]
    G = np.array(
        [[1, 0, 0], [0.5, 0.5, 0.5], [0.5, -0.5, 0.5], [0, 0, 1]], np.float32
    )
    U = np.einsum("jm,bdmcf->bcdjf", G, memW)  # [B, Cin, dy, j, F]
    U = np.ascontiguousarray(U, dtype=bf).reshape(B, Cin, KH * 4 * F)
    # input transform V_j over column windows of the zero-padded image
    Xp = np.zeros((B, RP, Wd + 2, Cin), np.float32)
    Xp[:, 1 : H + 1, 1 : Wd + 1] = X
    d0 = Xp[:, :, 0 : 2 * NP : 2]
    d1 = Xp[:, :, 1 : 2 * NP : 2]
    d2 = Xp[:, :, 2 : 2 * NP + 2 : 2]
    d3 = Xp[:, :, 3 : 2 * NP + 3 : 2]
    V = np.stack([d0 - d2, d1 + d2, d2 - d1, d1 - d3], axis=1)  # [B,4,RP,NP,Cin]
    V = np.ascontiguousarray(V.transpose(0, 4, 1, 2, 3), dtype=bf)
    V = V.reshape(B, Cin, 4 * VROW)
    # bias with F on partitions: mb[b, f', half]
    mbias = (bias[None] * Berr).astype(np.float32)  # [B, F]
    mbias = np.ascontiguousarray(mbias.reshape(B, 2, 128).transpose(0, 2, 1))
    return V, U, mbias


def _postprocess(y_cores):
    # y per core: [BPC, half, chunk, k, F-half, 16 rows, 32 pairs]
    out = np.concatenate(y_cores, axis=0)  # [B, 2, NCH, 2, 128, CHP]
    out = out.reshape(B, 2, NCH, 2, 128, 16, NP).astype(np.float32)
    # out[b, 16c+r, 2p+k, 128h+f] = y[b, h, c, k, f, r, p]
    out = out.transpose(0, 2, 5, 6, 3, 1, 4).reshape(B, H, Wd, F)
    return np.ascontiguousarray(out)


def kernel(X, W, bias, Werr, Berr):
    global _compiled
    from concourse.bass_utils import run_bass_kernel_spmd

    if _compiled is None:
        _compiled = _build_bass()
    nc = _compiled

    V, U, mbias = _prep_inputs(X, W, bias, Werr, Berr)
    in_maps = [
        {
            "vp": V[c * BPC : (c + 1) * BPC],
            "wm": U[c * BPC : (c + 1) * BPC],
            "mb": mbias[c * BPC : (c + 1) * BPC],
        }
        for c in range(NCORES)
    ]
    res = run_bass_kernel_spmd(nc, in_maps, core_ids=list(range(NCORES)))
    return _postprocess([r["y"] for r in res.results])
